# revision 20
# baseline (speedup 1.0000x reference)
"""Trainium2 Bass kernel: disentangled (DeBERTa-style) attention.

Full inputs in, full output out. Sharding: data-parallel over batch (4) x
tensor-parallel over head-groups (2) = 8 cores. Core c handles batch c//2,
heads (c%2)*6 .. +6. The relative-position tensors are replicated.

Key algebraic facts exploited:
  * P = table[rel] has only 513 distinct rows (rel depends on j-i only), so
    qr/kr = (P @ Wp) collapses to tableW = table @ Wp plus an index map.
  * c_p[i,s] = qc[i] . tableK[r],  c_r[i,s] = (tableQ[r] . kc_sum), with
    r = clip(i-s+256, 0, 512).  Both fold into one per-head strip
    CPc[i, r] = qc[i] . tableK[r] + cr[r]  of shape [S, 513].
  * The score contribution extra[i,s] = CPc[i, r(i,s)] is a Toeplitz skew of
    that strip: materialize a padded, reversed strip CPcE[i, u] (u in [0,768))
    in DRAM, then read 128x640 parallelogram tiles with a DMA access pattern
    whose partition stride is (768-1) elements -- each SBUF partition gets a
    contiguous run, so the DMA runs at line rate.  PE transpose-matmuls
    accumulate those tiles into the transposed score PSUM.  Fully saturated
    blocks (|i-s| > 383) are rank-1 and handled by K=1 matmuls.

Score layout is transposed ([s on partitions, i free]) so that attn@V needs
no transposes: out_raw[i,d] = sum_s exp[s,i] V[s,d] contracts s on the
partition dim, the softmax denominator rides along as a ones-column of V,
and normalization is a per-partition tensor_scalar.
"""

import math
from contextlib import ExitStack

import ml_dtypes
import numpy as np

import concourse.bass as bass
from concourse import bacc
import concourse.mybir as mybir
import concourse.tile as tile
from concourse.bass_utils import run_bass_kernel_spmd
from concourse.masks import make_identity

f32 = mybir.dt.float32
f32r = mybir.dt.float32r
bf16 = mybir.dt.bfloat16

B, S, D = 4, 1024, 768
NH, DH, KC = 12, 64, 256
HPC = NH // 2          # heads per core = 6
DG = HPC * DH          # 384 head-dims per core
W_CPE = 768            # padded skew strip width (127 | 513 | 128)
NCORES = 8

LAST_RESULT = None     # BassKernelResults of the most recent run (for tests)

MM_DT = f32r           # dtype for the big matmuls (f32r = full-rate fp32)
TR_DT = f32r           # dtype for PE transposes


def _v(ap, dt=None):
    return ap


def _sat_ranges(J):
    """Fully saturated column ranges of transposed-score block-row J.

    Returns (sat_row, lo, hi) triples: sat_row 0 => r=512 (i-s >= 257),
    sat_row 1 => r=0 (i-s <= -257).  Ranges are split at the 512-column PSUM
    bank boundary.
    """
    out = []
    lo = 128 * (J + 3)           # i >= 128*(J+3)  -> r = 512
    if lo < S:
        for b0 in (0, 512):
            a, b = max(lo, b0), min(S, b0 + 512)
            if a < b:
                out.append((0, a, b))
    hi = 128 * (J - 2)           # i < 128*(J-2)   -> r = 0
    if hi > 0:
        for b0 in (0, 512):
            a, b = max(0, b0), min(hi, b0 + 512)
            if a < b:
                out.append((1, a, b))
    return out


def build_bass():
    nc = bacc.Bacc("TRN2", target_bir_lowering=False)

    xtb = nc.dram_tensor("xtb", [D, S], bf16, kind="ExternalInput")
    wq = nc.dram_tensor("wq", [D, DG], bf16, kind="ExternalInput")
    wk = nc.dram_tensor("wk", [D, DG], bf16, kind="ExternalInput")
    wv = nc.dram_tensor("wv", [D, DG], bf16, kind="ExternalInput")
    bq = nc.dram_tensor("bq", [1, DG], bf16, kind="ExternalInput")
    bk = nc.dram_tensor("bk", [1, DG], bf16, kind="ExternalInput")
    bv = nc.dram_tensor("bv", [1, DG], bf16, kind="ExternalInput")
    cw = nc.dram_tensor("cw", [DG, D], bf16, kind="ExternalInput")
    tpad = nc.dram_tensor("tpad", [DH, W_CPE], bf16, kind="ExternalInput")
    wpq = nc.dram_tensor("wpq", [DH, DH], bf16, kind="ExternalInput")
    wpk = nc.dram_tensor("wpk", [DH, DH], bf16, kind="ExternalInput")
    mb = nc.dram_tensor("mb", [S], f32, kind="ExternalInput")
    out = nc.dram_tensor("out", [S, D], f32, kind="ExternalOutput")
    cpe = nc.dram_tensor("cpe", [HPC, S, W_CPE], bf16)  # skew strip scratch

    with tile.TileContext(nc) as tc, ExitStack() as ex:
        const = ex.enter_context(tc.tile_pool(name="const", bufs=1))
        persist = ex.enter_context(tc.tile_pool(name="persist", bufs=1))

        ident = const.tile([128, 128], f32, name="ident")
        make_identity(nc, ident[:])
        ident_b = const.tile([128, 128], bf16, name="ident_b")
        make_identity(nc, ident_b[:])
        zeros_b = const.tile([1, 128], bf16, name="zeros_b")
        nc.vector.memset(zeros_b[:], 0.0)
        ones_f = const.tile([1, 512], bf16, name="ones_f")
        nc.vector.memset(ones_f[:], 1.0)
        ones_b = const.tile([1, 128], bf16, name="ones_b")
        nc.vector.memset(ones_b[:], 1.0)
        mb_sb = const.tile([128, 8], f32, name="mb_sb")
        nc.sync.dma_start(out=mb_sb[:], in_=bass.AP(mb, 0, [[1, 128], [128, 8]]))

        QT = [persist.tile([128, S], bf16, name=f"QT{t}") for t in range(3)]
        KT = [persist.tile([128, S], bf16, name=f"KT{t}") for t in range(3)]
        Vb = [persist.tile([128, HPC * 65], bf16, name=f"Vb{j}") for j in range(8)]
        cws = [persist.tile([128, D], bf16, name=f"cw{c}") for c in range(3)]
        TQp = persist.tile([DH, W_CPE], bf16, name="TQp")
        kcs = [persist.tile([128, 1], f32, name=f"kcs{t}") for t in range(3)]
        satT = [[persist.tile([1, S], bf16, name=f"satT{h}_{p}") for p in range(2)]
                for h in range(HPC)]

        for c in range(3):
            nc.sync.dma_start(out=cws[c][:], in_=cw[128 * c:128 * (c + 1), :])

        # ---------------- Phase A: QKV projection + tables ----------------
        ab = ExitStack()
        wload = ab.enter_context(tc.tile_pool(name="wload", bufs=1))
        abp = ab.enter_context(tc.tile_pool(name="abp", bufs=1))

        xbs, wqs, wks, wvs = [], [], [], []
        for t in range(6):
            xb = wload.tile([128, S], bf16, name=f"xbt{t}")
            nc.sync.dma_start(out=xb[:], in_=xtb[128 * t:128 * (t + 1), :])
            xbs.append(xb)
        for nm, dram, dt_, lst in (("wq", wq, bf16, wqs), ("wk", wk, bf16, wks),
                                   ("wv", wv, bf16, wvs)):
            for t in range(6):
                w = wload.tile([128, DG], dt_, name=f"{nm}{t}")
                nc.sync.dma_start(out=w[:], in_=dram[128 * t:128 * (t + 1), :])
                lst.append(w)
        bq_sb = wload.tile([1, DG], bf16, name="bq_sb")
        bk_sb = wload.tile([1, DG], bf16, name="bk_sb")
        bv_sb = wload.tile([1, DG], bf16, name="bv_sb")
        nc.sync.dma_start(out=bq_sb[:], in_=bq[:, :])
        nc.sync.dma_start(out=bk_sb[:], in_=bk[:, :])
        nc.sync.dma_start(out=bv_sb[:], in_=bv[:, :])
        tpad_sb = wload.tile([DH, W_CPE], bf16, name="tpad_sb")
        nc.sync.dma_start(out=tpad_sb[:], in_=tpad[:, :])
        wpq_sb = wload.tile([DH, DH], bf16, name="wpq_sb")
        nc.sync.dma_start(out=wpq_sb[:], in_=wpq[:, :])
        wpk_sb = wload.tile([DH, DH], bf16, name="wpk_sb")
        nc.sync.dma_start(out=wpk_sb[:], in_=wpk[:, :])

        tc.strict_bb_all_engine_barrier()
        with tc.tile_pool(name="psA", space="PSUM", bufs=4) as psA, \
             tc.tile_pool(name="psT", space="PSUM", bufs=2) as psT:
            # Q^T and K^T: out[d_chunk, s] = sum_D W[D, d] x^T[D, s]
            for dst, wlist, brow in ((QT, wqs, bq_sb), (KT, wks, bk_sb)):
                for m in range(3):
                    for n2 in range(2):
                        ps = psA.tile([128, 512], f32, name="psA_t", tag="psA")
                        for kk in range(6):
                            nc.tensor.matmul(
                                ps[:], _v(wlist[kk][:, 128 * m:128 * (m + 1)]),
                                xbs[kk][:, 512 * n2:512 * (n2 + 1)],
                                start=(kk == 0), stop=False)
                        nc.tensor.matmul(
                            ps[:], _v(brow[0:1, 128 * m:128 * (m + 1)]),
                            _v(ones_f[0:1, :]), start=False, stop=True)
                        if dst is KT:
                            nc.scalar.copy(dst[m][:, 512 * n2:512 * (n2 + 1)], ps[:])
                        else:
                            nc.vector.tensor_copy(dst[m][:, 512 * n2:512 * (n2 + 1)], ps[:])
            # V: out[s_chunk, d] = sum_D x^T[D, s] Wv[D, d]; pitch-65 bf16 + ones col
            for j in range(8):
                ps = psA.tile([128, DG], f32, name="psA_v", tag="psA")
                for kk in range(6):
                    nc.tensor.matmul(
                        ps[:], xbs[kk][:, 128 * j:128 * (j + 1)], wvs[kk][:],
                        start=(kk == 0), stop=False)
                nc.tensor.matmul(ps[:], _v(ones_f[0:1, 0:128]), _v(bv_sb[0:1, :]),
                                 start=False, stop=True)
                vdst = Vb[j][:].rearrange("p (h c) -> p h c", h=HPC)
                nc.vector.tensor_copy(vdst[:, :, 0:64],
                                      ps[:].rearrange("p (h c) -> p h c", h=HPC))
                nc.vector.memset(vdst[:, :, 64:65], 1.0)

            # kc_sum (per 2-head tile): reduce K^T along free dim
            for t in range(3):
                nc.vector.tensor_reduce(kcs[t][:], KT[t][:],
                                        axis=mybir.AxisListType.X,
                                        op=mybir.AluOpType.add)

            # tableW strips: TKp = Wp_k^T @ tpad, TQp = Wp_q^T @ tpad
            TKp_sb = wload.tile([DH, W_CPE], bf16, name="TKp_sb")
            for wsb, dsts in ((wpk_sb, TKp_sb), (wpq_sb, TQp)):
                ps = psT.tile([DH, W_CPE], f32, name="psT_t", tag="psT")
                nc.tensor.matmul(ps[:, 0:512], _v(wsb[:]), _v(tpad_sb[:, 0:512]),
                                 start=True, stop=True)
                nc.tensor.matmul(ps[:, 512:W_CPE], _v(wsb[:]), _v(tpad_sb[:, 512:W_CPE]),
                                 start=True, stop=True)
                nc.vector.tensor_copy(dsts[:], ps[:])

            # Augmented per-head operands:
            #   QTaug[h] = [ones; Q^T rows of head h]          [65, S]
            #   TKaug[h] = [cr_pad row; TKp]                   [65, W_CPE]

            tc.strict_bb_all_engine_barrier()
            QTaug = [abp.tile([65, S], bf16, name=f"QTaug{h}") for h in range(HPC)]
            TKaug = [abp.tile([65, W_CPE], bf16, name=f"TKaug{h}") for h in range(HPC)]
            kc_col = [abp.tile([DH, 1], bf16, name=f"kc{h}") for h in range(HPC)]
            satcols = [abp.tile([65, 2], bf16, name=f"satc{h}") for h in range(HPC)]
            crh_sb = [abp.tile([1, W_CPE], bf16, name=f"crh{h}") for h in range(HPC)]
            for h in range(HPC):
                t, r = divmod(h, 2)
                nc.sync.dma_start(out=QTaug[h][0:64, :], in_=QT[t][64 * r:64 * r + 64, :])
                nc.vector.memset(QTaug[h][64:65, :], 1.0)
                nc.vector.tensor_copy(TKaug[h][0:64, :], TKp_sb[:])
                nc.gpsimd.dma_start(out=kc_col[h][:], in_=kcs[t][64 * r:64 * r + 64, 0:1])
            for h in range(HPC):
                ps = psT.tile([1, W_CPE], f32, name="psT_cr", tag="psT")
                nc.tensor.matmul(ps[:, 0:512], kc_col[h][:], TQp[:, 0:512],
                                 start=True, stop=True)
                nc.tensor.matmul(ps[:, 512:W_CPE], kc_col[h][:], TQp[:, 512:W_CPE],
                                 start=True, stop=True)
                nc.vector.tensor_copy(crh_sb[h][:], ps[:])
                nc.sync.dma_start(out=TKaug[h][64:65, :], in_=crh_sb[h][:])
            # saturated-value rows: satT[h][p] = satcols[h][:,p]^T @ QTaug[h]
            for h in range(HPC):
                nc.vector.tensor_copy(satcols[h][:, 0:1], TKaug[h][:, 127:128])
                nc.vector.tensor_copy(satcols[h][:, 1:2], TKaug[h][:, 639:640])
                for p in range(2):
                    ps = psT.tile([1, S], f32, name="psT_sat", tag="psT")
                    for n2 in range(2):
                        nc.tensor.matmul(ps[:, 512 * n2:512 * (n2 + 1)],
                                         satcols[h][:, p:p + 1],
                                         QTaug[h][:, 512 * n2:512 * (n2 + 1)],
                                         start=True, stop=True)
                    nc.vector.tensor_copy(satT[h][p][:], ps[:])

        # ---------------- Phase B: CPcE strips -> DRAM --------------------
        tc.strict_bb_all_engine_barrier()
        with tc.tile_pool(name="psB", space="PSUM", bufs=3) as psB, \
             tc.tile_pool(name="cpool", bufs=4) as cpool:
            for h in range(HPC):
                for I in range(8):
                    ps = psB.tile([128, W_CPE], f32, name="psB_t", tag="psB")
                    lhs = QTaug[h][:, 128 * I:128 * (I + 1)]
                    nc.tensor.matmul(ps[:, 0:512], _v(lhs), _v(TKaug[h][:, 0:512]),
                                     start=True, stop=True)
                    nc.tensor.matmul(ps[:, 512:W_CPE], _v(lhs), _v(TKaug[h][:, 512:W_CPE]),
                                     start=True, stop=True)
                    ct = cpool.tile([128, W_CPE], bf16, name="cpe_t")
                    if I % 2:
                        nc.scalar.copy(ct[:], ps[:])
                    else:
                        nc.vector.tensor_copy(ct[:], ps[:])
                    nc.sync.dma_start(out=cpe[h, 128 * I:128 * (I + 1), :], in_=ct[:])
        ab.close()  # frees xT/W/QTaug/TKaug sbuf

        # ------------- Phase C/D: scores^T, softmax, attn@V ---------------
        headout = [persist.tile([128, DG], bf16, name=f"ho{ic}") for ic in range(8)]
        tc.strict_bb_all_engine_barrier()
        with tc.tile_pool(name="psC", space="PSUM", bufs=3) as psC, \
             tc.tile_pool(name="psD", space="PSUM", bufs=2) as psD, \
             tc.tile_pool(name="srp", bufs=12) as srp, \
             tc.tile_pool(name="expp", bufs=20) as expp, \
             tc.tile_pool(name="rcp", bufs=4) as rcp:
            for hp in range(3):
                heads = (2 * hp, 2 * hp + 1)
                SRs = {h: {} for h in heads}
                expT = {}
                for J in range(8):
                    new_Is = range(0, min(8, 3)) if J == 0 else \
                        (range(J + 2, J + 3) if J + 2 < 8 else range(0))
                    new_srs = []
                    for h in heads:
                        for I in new_Is:
                            sr = srp.tile([128, 640], f32, name="sr")
                            s_lo = max(0, 128 * (I - 2))
                            s_hi = min(S, 128 * (I + 3))
                            s0 = s_lo - 128 * (I - 2)
                            Wd = s_hi - s_lo
                            off = (h * S + 128 * I) * W_CPE + 127 + s0
                            src = bass.AP(cpe, off, [[W_CPE - 1, 128], [1, Wd]])
                            nc.gpsimd.dma_start(out=sr[:, s0:s0 + Wd], in_=src)
                            SRs[h][I] = sr
                            new_srs.append((h, sr, s0))
                    scs = {}
                    for h in heads:
                        t, r = divmod(h, 2)
                        sc = psC.tile([128, S], f32, name="sc")
                        scs[h] = sc
                        tp = (64 * r, 0) if r else None
                        # plan per-bank op lists so the last op per bank gets stop=True
                        bank_ops = {0: [], 1: []}
                        for n2 in range(2):
                            bank_ops[n2].append(("cc", n2))
                        for I in range(max(0, J - 2), min(8, J + 3)):
                            bank_ops[I // 4].append(("tr", I))
                        for row, lo, hi in _sat_ranges(J):
                            bank_ops[lo // 512].append(("sat", (row, lo, hi)))
                        last = {b: ops[-1] for b, ops in bank_ops.items()}
                        for n2 in range(2):
                            nc.tensor.matmul(
                                sc[:, 512 * n2:512 * (n2 + 1)],
                                _v(KT[t][64 * r:64 * r + 64, 128 * J:128 * (J + 1)]),
                                _v(QT[t][64 * r:64 * r + 64, 512 * n2:512 * (n2 + 1)]),
                                start=True, stop=(last[n2] == ("cc", n2)),
                                tile_position=tp)
                        scs[h] = (sc, last)
                    for h2, srt, s0a in new_srs:
                        sc2 = scs[h2][0]
                        nc.tensor.matmul(sc2[:, 0:2], zeros_b[0:1, :],
                                         srt[0:1, s0a:s0a + 1].bitcast(bf16),
                                         start=False, stop=False)
                    for h in heads:
                        sc, last = scs[h]
                        # banded blocks: transpose-accumulate skewed strip tiles
                        for I in range(max(0, J - 2), min(8, J + 3)):
                            dlt = I - J
                            nc.tensor.matmul(
                                sc[:, 128 * I:128 * (I + 1)],
                                SRs[h][I][:, 128 * (2 - dlt):128 * (3 - dlt)],
                                ident[:],
                                is_transpose=True, start=False,
                                stop=(last[I // 4] == ("tr", I)))
                        # fully saturated blocks: rank-1
                        for row, lo, hi in _sat_ranges(J):
                            nc.tensor.matmul(
                                sc[:, lo:hi], ones_b[0:1, :],
                                satT[h][row][0:1, lo:hi],
                                start=False,
                                stop=(last[lo // 512] == ("sat", (row, lo, hi))))
                        et = expp.tile([128, S], bf16, name="et")
                        nc.scalar.activation(et[:], sc[:],
                                             mybir.ActivationFunctionType.Exp,
                                             bias=mb_sb[:, J:J + 1], scale=1.0)
                        expT[(h, J)] = et
                # attn@V with fused denominator column
                for h in heads:
                    for ic in range(8):
                        ov = psD.tile([128, 65], f32, name="ov")
                        for J in range(8):
                            nc.tensor.matmul(
                                ov[:], expT[(h, J)][:, 128 * ic:128 * (ic + 1)],
                                Vb[J][:, 65 * h:65 * (h + 1)],
                                start=(J == 0), stop=(J == 7))
                        rc = rcp.tile([128, 1], f32, name="rc")
                        nc.vector.reciprocal(rc[:], ov[:, 64:65])
                        nc.vector.tensor_scalar(
                            out=headout[ic][:, 64 * h:64 * (h + 1)],
                            in0=ov[:, 0:64], scalar1=rc[:], scalar2=None,
                            op0=mybir.AluOpType.mult)

        # ---------------- Phase E: transpose heads + c_proj ----------------
        tc.strict_bb_all_engine_barrier()
        with tc.tile_pool(name="psE", space="PSUM", bufs=2) as psE, \
             tc.tile_pool(name="outp", bufs=4) as op:
            hoT = [persist.tile([128, S], bf16, name=f"hoT{c}") for c in range(3)]
            for ic in range(8):
                for c in range(3):
                    tp = psE.tile([128, 128], bf16, name="tp", tag="tp")
                    nc.tensor.matmul(tp[:],
                                     headout[ic][:, 128 * c:128 * (c + 1)],
                                     ident_b[:],
                                     is_transpose=True, start=True, stop=True)
                    nc.vector.tensor_copy(hoT[c][:, 128 * ic:128 * (ic + 1)], tp[:])
            for ic in range(8):
                ot = op.tile([128, D], f32, name="ot")
                for n2 in range(2):
                    pc = psE.tile([128, 384], f32, name="pc", tag="pc")
                    for c in range(3):
                        nc.tensor.matmul(pc[:], _v(hoT[c][:, 128 * ic:128 * (ic + 1)]),
                                         _v(cws[c][:, 384 * n2:384 * (n2 + 1)]),
                                         start=(c == 0), stop=(c == 2))
                    if ic % 2:
                        nc.scalar.copy(ot[:, 384 * n2:384 * (n2 + 1)], pc[:])
                    else:
                        nc.vector.tensor_copy(ot[:, 384 * n2:384 * (n2 + 1)], pc[:])
                nc.sync.dma_start(out=out[128 * ic:128 * (ic + 1), :], in_=ot[:])

    nc.compile()
    return nc


_NC_CACHE = None


def _get_nc():
    global _NC_CACHE
    if _NC_CACHE is None:
        _NC_CACHE = build_bass()
    return _NC_CACHE


def make_in_maps(x, attention_mask, Wc_w, Wc_b, Wp_w, table, cproj_w):
    x = np.asarray(x, np.float32)
    attention_mask = np.asarray(attention_mask)
    Wc_w = np.asarray(Wc_w, np.float32)
    Wc_b = np.asarray(Wc_b, np.float32)
    Wp_w = np.asarray(Wp_w, np.float32)
    table = np.asarray(table, np.float32)
    cproj_w = np.asarray(cproj_w, np.float32)

    scale = 1.0 / math.sqrt(DH)
    idx = np.clip(639 - np.arange(W_CPE), 0, 512)
    tpad_np = np.ascontiguousarray(table.T[:, idx])
    wpq_np = np.ascontiguousarray(Wp_w[:, 0:DH]) * scale
    wpk_np = np.ascontiguousarray(Wp_w[:, DH:2 * DH])

    in_maps = []
    for c in range(NCORES):
        b, hg = divmod(c, 2)
        sl = slice(hg * DG, (hg + 1) * DG)
        bf = ml_dtypes.bfloat16
        xt_c = np.ascontiguousarray(x[b].T)
        in_maps.append({
            "xtb": xt_c.astype(bf),
            "wq": (np.ascontiguousarray(Wc_w[:, sl]) * scale).astype(bf),
            "wk": np.ascontiguousarray(Wc_w[:, D + hg * DG: D + (hg + 1) * DG]).astype(bf),
            "wv": np.ascontiguousarray(Wc_w[:, 2 * D + hg * DG: 2 * D + (hg + 1) * DG]).astype(bf),
            "bq": (Wc_b[sl] * scale).reshape(1, DG).astype(bf),
            "bk": Wc_b[D + hg * DG: D + (hg + 1) * DG].reshape(1, DG).astype(bf),
            "bv": Wc_b[2 * D + hg * DG: 2 * D + (hg + 1) * DG].reshape(1, DG).astype(bf),
            "cw": np.ascontiguousarray(cproj_w[sl, :]).astype(bf),
            "tpad": tpad_np.astype(bf),
            "wpq": wpq_np.astype(bf),
            "wpk": wpk_np.astype(bf),
            "mb": np.where(attention_mask[b] == 0, -1e9, 0.0).astype(np.float32),
        })
    return in_maps


def kernel(x, attention_mask, Wc_w, Wc_b, Wp_w, table, cproj_w, cproj_b,
           n_h, k, **_ignored):
    global LAST_RESULT
    assert int(n_h) == NH and int(k) == KC
    in_maps = make_in_maps(x, attention_mask, Wc_w, Wc_b, Wp_w, table, cproj_w)
    nc = _get_nc()
    res = run_bass_kernel_spmd(nc, in_maps, list(range(NCORES)))
    LAST_RESULT = res
    outs = res.results
    full = np.zeros((B, S, D), np.float32)
    for b in range(B):
        full[b] = outs[2 * b]["out"] + outs[2 * b + 1]["out"]
    full += np.asarray(cproj_b, np.float32)[None, None, :]
    return full
